# revision 22
# baseline (speedup 1.0000x reference)
"""AlgebraicTransformerLM on 8 trn2 NeuronCores (Bass/Tile), v3.

Sharding: DP=2 over batch x TP=4 over heads / d_ffn / vocab (cores 0-3 =
batch 0, 4-7 = batch 1). v3 is a scheduling rewrite of v2 targeting the
HAM half-clock throttle (v2 spent ~800us at K=4/8 because the PE idled
>3.4us inside every per-head score->softmax->AV chain):
  - Emission is a hand-woven interleave: FFN / out-proj / qkv matmuls and
    the NEXT sublayer's work are emitted between attention softmax rounds
    so the PE stream stays dense and the elementwise chains (which bound
    the softmax) run in their shadow.
  - Per-strip attention runs all 4 heads in lockstep rounds with the AV
    accumulation lagging LAG rounds behind the score matmuls, so no AV
    matmul ever head-blocks the in-order PE queue.
  - Causal masking via one gpsimd affine_select (fill=-100) on the fp16
    score copy instead of two gpsimd mask multiplies.
  - Row broadcasts (1/denominator, 1/mag) via gpsimd partition_broadcast
    into SBUF instead of PE ones-matmuls into PSUM; PSUM banks: 4 AV +
    3 general + 1 mag = 8.
  - wm loaded once per layer (v2 re-DMA'd it per strip); weight DMAs
    spread across the sync + scalar HWDGE queues.
  - lm_head: all 16 vocab blocks stay resident in SBUF (one fetch), the
    strip-0 token tiles of every block run before the final AR-dependent
    addnorm, logits stored fp16 (halves the output DMA).
  - Warmup AllReduce at kernel start absorbs the cold ncfw cost.
Host prep: fold norm weights/SCALE/0.5 into adjacent matmul weights,
precompute xn0 = norm(emb[ids]); logits returned fp16, cast on host.
"""
import contextlib
import math

import numpy as np

import concourse.bacc as bacc
import concourse.mybir as mybir
import concourse.tile as tile
from concourse.bass_utils import run_bass_kernel_spmd

F32 = mybir.dt.float32
F32R = mybir.dt.float32r
FP16 = mybir.dt.float16

B, T, V, D, H, L = 2, 1024, 32000, 1024, 16, 4
DFF = 2730
DH = D // H
SCALE = 1.0 / math.sqrt(DH)
EPS = 1e-6

NCORES = 8
TP = 4
HPC = H // TP               # heads per core (4)
FSH = 2 * DH * HPC          # q+k rows per core (512)
VSH = DH * HPC              # v rows per core (256)
DFF_SH = 768                # padded DFF shard (4*768 >= 2730)
NFT_FF = DFF_SH // 128      # 6
VOC_SH = V // TP            # vocab shard per core (8000)
NVS = (VOC_SH + 511) // 512         # 16 lm blocks
DT = D // 128               # 8
NSTRIP = T // 512           # 2
LAG = 2
RG = [[0, 1, 2, 3], [4, 5, 6, 7]]
ALIBI = [2.0 ** (-8.0 * (i + 1) / H) for i in range(H)]

_CACHE = {}

AF = mybir.ActivationFunctionType
ALU = mybir.AluOpType


def _causal_tk(s):
    return list(range((s + 1) * (512 // 128)))


def _pin_act_table(arch):
    """Make every activation resolve to natural_log_exp_and_others so the
    ACT table is loaded once instead of ping-ponging between sets."""
    from concourse.hw_specs import get_activation_tables

    tabs = get_activation_tables(arch)
    keep = "natural_log_exp_and_others"
    mine = {AF.Abs, AF.Copy, AF.Square, AF.Exp, AF.Ln}
    if keep not in tabs or not (mine <= tabs[keep]):
        return
    for name, funcs in tabs.items():
        if name != keep:
            funcs -= mine


def build_nc():
    nc = bacc.Bacc("TRN2", target_bir_lowering=False)
    _pin_act_table(nc.m.arch)

    x0T = nc.dram_tensor("x0T", [D, T], F32, kind="ExternalInput")
    xn0T = nc.dram_tensor("xn0T", [D, T], FP16, kind="ExternalInput")
    qaug = nc.dram_tensor("qaug", [HPC, 2, T], F32, kind="ExternalInput")
    kaug = nc.dram_tensor("kaug", [HPC, 2, T], F32, kind="ExternalInput")
    wqkT = nc.dram_tensor("wqkT", [L, D, FSH], FP16, kind="ExternalInput")
    wvT = nc.dram_tensor("wvT", [L, D, VSH], FP16, kind="ExternalInput")
    woT = nc.dram_tensor("woT", [L, VSH, D], FP16, kind="ExternalInput")
    wmT = nc.dram_tensor("wmT", [L, D, 2 * DFF_SH], FP16, kind="ExternalInput")
    w3T = nc.dram_tensor("w3T", [L, DFF_SH, D], FP16, kind="ExternalInput")
    membT = nc.dram_tensor("membT", [D, VOC_SH], FP16, kind="ExternalInput")
    logits = nc.dram_tensor("logits", [T, VOC_SH], FP16, kind="ExternalOutput")
    NCH = 2 * L * NSTRIP
    cc_in = [nc.dram_tensor(f"cc_in{i}", [D, 512], FP16) for i in range(NCH)]
    cc_out = [nc.dram_tensor(f"cc_out{i}", [D, 512], FP16) for i in range(NCH)]
    wu_in = nc.dram_tensor("wu_in", [1, 128], FP16)
    wu_out = nc.dram_tensor("wu_out", [1, 128], FP16)

    with tile.TileContext(nc) as tc, contextlib.ExitStack() as ctx:
        persist = ctx.enter_context(tc.tile_pool(name="persist", bufs=1))
        psAv = ctx.enter_context(tc.tile_pool(name="psAv", bufs=4, space="PSUM"))
        psA = ctx.enter_context(tc.tile_pool(name="psA", bufs=3, space="PSUM"))
        psN = ctx.enter_context(tc.tile_pool(name="psN", bufs=1, space="PSUM"))

        # ---- warmup collective: absorb the cold ncfw/SPAD cost -----------
        wu = persist.tile([1, 128], FP16, tag="wu")
        nc.vector.memset(wu[:], 0.0)
        nc.gpsimd.dma_start(wu_in[:], wu[:])
        nc.gpsimd.collective_compute("AllReduce", ALU.add, ins=[wu_in[:]],
                                     outs=[wu_out[:]], replica_groups=RG)

        xn = persist.tile([128, DT, T], FP16, tag="xn")
        nc.sync.dma_start(xn[:], xn0T[:].rearrange("(dt p) t -> p dt t", p=128))
        # x (f32, 4MB) rides the gpsimd SWDGE queue so it doesn't delay the
        # layer-0 weight loads on the two HWDGE queues; first use is the
        # first addnorm ~70us in.
        x = persist.tile([128, DT, T], F32, tag="x")
        nc.gpsimd.dma_start(x[:], x0T[:].rearrange("(dt p) t -> p dt t", p=128))

        ocf = persist.tile([128, 1], F32, tag="ones_colf")
        nc.vector.memset(ocf[:], 1.0)
        ones_colb = persist.tile([128, 1], FP16, tag="ones_colb")
        nc.vector.tensor_copy(ones_colb[:], ocf[:])
        ones_b = persist.tile([128, 1], F32, tag="ones_bias")
        nc.vector.memset(ones_b[:], 1.0)

        def sigpipe(idx, spool, sc, w4_out, n, masked):
            """w4 = ((1+u)/2*2)^4-ish weights in fp16 from the score psum.
            Masked (diagonal) tiles: future entries forced to -100 via one
            gpsimd affine_select => w4 underflows to exactly 0 in fp16."""
            act_var = (idx % 4 == 1)
            if masked:
                sb0 = spool.tile([128, 512], FP16, tag="sb")
                nc.scalar.activation(sb0[:, :n], sc[:, :n], AF.Copy, scale=1.0)
                sb = spool.tile([128, 512], FP16, tag="sbm")
                nc.gpsimd.affine_select(sb[:, :n], sb0[:, :n],
                                        pattern=[[1, n]], base=0,
                                        channel_multiplier=-1,
                                        compare_op=ALU.is_ge, fill=-100.0)
                src_s = sb
                a = spool.tile([128, 512], FP16, tag="aa")
                nc.scalar.activation(a[:, :n], sb[:, :n], AF.Abs, scale=1.0)
            else:
                src_s = sc
                a = spool.tile([128, 512], FP16, tag="aa")
                nc.scalar.activation(a[:, :n], sc[:, :n], AF.Abs, scale=1.0)
            if act_var:
                ln = spool.tile([128, 512], FP16, tag="rr")
                nc.scalar.activation(ln[:, :n], a[:, :n], AF.Ln, bias=1.0,
                                     scale=1.0)
                r = spool.tile([128, 512], FP16, tag="rb")
                nc.scalar.activation(r[:, :n], ln[:, :n], AF.Exp, scale=-1.0)
            else:
                d = spool.tile([128, 512], F32, tag="df")
                nc.vector.tensor_scalar(d[:, :n], a[:, :n], scalar1=1.0,
                                        scalar2=None, op0=ALU.add,
                                        op1=ALU.bypass)
                r = spool.tile([128, 512], F32, tag="rf")
                nc.vector.reciprocal_approx_fast(r[:, :n], d[:, :n])
            u = spool.tile([128, 512], FP16, tag="uu")
            nc.vector.tensor_tensor(u[:, :n], src_s[:, :n], r[:, :n], ALU.mult)
            w2 = spool.tile([128, 512], FP16, tag="w2")
            nc.scalar.activation(w2[:, :n], u[:, :n], AF.Square,
                                 bias=ones_b[:], scale=1.0)
            if idx % 2 == 0:
                nc.gpsimd.tensor_tensor(w4_out[:, :n], w2[:, :n], w2[:, :n],
                                        ALU.mult)
            else:
                nc.vector.tensor_tensor(w4_out[:, :n], w2[:, :n], w2[:, :n],
                                        ALU.mult)

        npool = ctx.enter_context(tc.tile_pool(name="npool", bufs=2))
        rowpool = ctx.enter_context(tc.tile_pool(name="rowpool", bufs=3))
        reppool = ctx.enter_context(tc.tile_pool(name="reppool", bufs=1))

        def addnorm(idx, s):
            """x[:, :, strip] += AR chunk idx (SWDGE accum); xn strip =
            normed x (fp16)."""
            sl = slice(s * 512, (s + 1) * 512)
            nc.gpsimd.dma_start(
                x[:, :, sl],
                cc_out[idx][:].rearrange("(dt p) t -> p dt t", p=128),
                accum_op=ALU.add)
            mag = psN.tile([1, 512], F32, tag="mag", name=f"mag{idx}")
            for q in range(4):
                xa = npool.tile([128, 2, 512], FP16, tag="nabs")
                nc.scalar.activation(xa[:], x[:, 2 * q:2 * q + 2, sl],
                                     AF.Abs, scale=1.0)
                for i in range(2):
                    dt = 2 * q + i
                    nc.tensor.matmul(mag[:], ones_colb[:], xa[:, i],
                                     start=(dt == 0), stop=(dt == DT - 1),
                                     skip_group_check=True)
            md = rowpool.tile([1, 512], F32, tag="row")
            nc.vector.tensor_scalar(md[:], mag[:], scalar1=1.0 / D,
                                    scalar2=EPS, op0=ALU.mult, op1=ALU.add)
            mr = rowpool.tile([1, 512], F32, tag="row")
            nc.vector.reciprocal_approx_fast(mr[:], md[:])
            mrb = rowpool.tile([1, 512], FP16, tag="rowb")
            nc.vector.tensor_copy(mrb[:], mr[:])
            rep = reppool.tile([128, 512], FP16, tag="repb")
            nc.gpsimd.partition_broadcast(rep[:], mrb[:])
            for dt in range(DT):
                nc.vector.tensor_tensor(xn[:, dt, sl], x[:, dt, sl], rep[:],
                                        ALU.mult)

        # ================= generators =====================================
        with tc.tile_pool(name="wpool", bufs=1) as wpool, \
             tc.tile_pool(name="apool", bufs=1) as apool, \
             tc.tile_pool(name="w4pool", bufs=10) as w4pool, \
             tc.tile_pool(name="sbpool", bufs=3) as sbpool, \
             tc.tile_pool(name="sbmpool", bufs=2) as sbmpool, \
             tc.tile_pool(name="scrpool", bufs=3) as scrpool, \
             tc.tile_pool(name="scr2pool", bufs=2) as scr2pool, \
             tc.tile_pool(name="scrfpool", bufs=2) as scrfpool, \
             tc.tile_pool(name="ewpool", bufs=2) as ewpool, \
             tc.tile_pool(name="hsbpool", bufs=1) as hsbpool, \
             tc.tile_pool(name="dlpool", bufs=2) as dlpool:

            class SP:
                def tile(self, shape, dtype, tag):
                    pool = {"sb": sbpool, "sbm": sbmpool, "aa": scrpool,
                            "rr": scr2pool, "rb": scr2pool, "uu": scrpool,
                            "w2": scrpool, "df": scrfpool, "rf": scrfpool,
                            "gb": ewpool, "tt": ewpool, "m2": ewpool,
                            "vb": ewpool}[tag]
                    SPc[0] += 1
                    return pool.tile(shape, dtype, tag=tag,
                                     name=f"{tag}_{SPc[0]}")
            SPc = [0]
            spool = SP()

            wt = {}

            def load_qkvo(l):
                wt["wqk"] = wpool.tile([128, DT, FSH], FP16, tag="wqk",
                                       name=f"wqk{l}")
                nc.sync.dma_start(wt["wqk"][:],
                                  wqkT[l].rearrange("(dt p) f -> p dt f",
                                                    p=128))
                wt["wv"] = wpool.tile([128, DT, VSH], FP16, tag="wv",
                                      name=f"wv{l}")
                nc.scalar.dma_start(wt["wv"][:],
                                    wvT[l].rearrange("(dt p) f -> p dt f",
                                                     p=128))
                wt["wo"] = wpool.tile([128, 2, D], FP16, tag="wo",
                                      name=f"wo{l}")
                nc.scalar.dma_start(wt["wo"][:],
                                    woT[l].rearrange("(pp p) f -> p pp f",
                                                     p=128))

            def load_wm(l):
                wt["wm"] = wpool.tile([128, DT, 2 * DFF_SH], FP16, tag="wm",
                                      name=f"wm{l}")
                nc.sync.dma_start(wt["wm"][:],
                                  wmT[l].rearrange("(dt p) f -> p dt f",
                                                   p=128))

            def load_w3(l):
                wt["w3"] = wpool.tile([128, NFT_FF, D], FP16, tag="w3",
                                      name=f"w3{l}")
                nc.scalar.dma_start(wt["w3"][:],
                                    w3T[l].rearrange("(ft p) f -> p ft f",
                                                     p=128))

            qa = {}
            ka = {}
            vaug = {}
            asb = {}
            hsbs = {}

            def alloc_attn(l):
                qa[0] = [apool.tile([66, T], F32R, tag=f"qa{h}",
                                    name=f"qa{h}_{l}") for h in range(HPC)]
                ka[0] = [apool.tile([66, T], F32R, tag=f"ka{h}",
                                    name=f"ka{h}_{l}") for h in range(HPC)]
                for h in range(HPC):
                    nc.sync.dma_start(qa[0][h][64:66, :],
                                      qaug[h].bitcast(F32R))
                    nc.sync.dma_start(ka[0][h][64:66, :],
                                      kaug[h].bitcast(F32R))
                vaug[0] = apool.tile([128, DT, HPC * 65], FP16, tag="vaug",
                                     name=f"vaug{l}")
                nc.vector.memset(vaug[0][:], 1.0)
                asb[0] = apool.tile([128, 2, T], FP16, tag="asb",
                                    name=f"asb{l}")

            def units_qk(l, s):
                sl = slice(s * 512, (s + 1) * 512)
                for ft in range(4):
                    qk, pair = ft // 2, ft % 2
                    ps = psA.tile([128, 512], F32, tag="ps",
                                  name=f"qkps{l}_{ft}_{s}")
                    for dt in range(DT):
                        nc.tensor.matmul(
                            ps[:], wt["wqk"][:, dt, ft * 128:(ft + 1) * 128],
                            xn[:, dt, sl], start=(dt == 0),
                            stop=(dt == DT - 1))
                    tgt = qa[0] if qk == 0 else ka[0]
                    nc.scalar.activation(tgt[2 * pair][0:64, sl], ps[0:64, :],
                                         AF.Copy, scale=1.0)
                    nc.scalar.activation(tgt[2 * pair + 1][0:64, sl],
                                         ps[64:128, :], AF.Copy, scale=1.0)
                    yield

            def units_v(l, s):
                for tt in range(s * 4, s * 4 + 4):
                    ps = psA.tile([128, 512], F32, tag="ps",
                                  name=f"vps{l}_{tt}")
                    for dt in range(DT):
                        nc.tensor.matmul(ps[:, 0:VSH],
                                         xn[:, dt, tt * 128:(tt + 1) * 128],
                                         wt["wv"][:, dt], start=(dt == 0),
                                         stop=(dt == DT - 1))
                    for h in range(HPC):
                        dst = vaug[0][:, tt, h * 65:h * 65 + 64]
                        if h % 2 == 0:
                            nc.vector.tensor_copy(dst, ps[:, h * 64:(h + 1) * 64])
                        else:
                            nc.scalar.activation(dst, ps[:, h * 64:(h + 1) * 64],
                                                 AF.Copy, scale=1.0)
                    yield

            def units_scores(l, s):
                tks = _causal_tk(s)
                avs = [psAv.tile([65, 512], F32, tag="av",
                                 name=f"av{l}_{h}_{s}") for h in range(HPC)]
                ring = {}
                nrounds = len(tks)
                for i in range(nrounds + LAG):
                    if i < nrounds:
                        tk = tks[i]
                        col0 = max(0, tk * 128 - s * 512)
                        masked = tk * 128 + 127 > s * 512
                        n = 512 - col0
                        for h in range(HPC):
                            sc = psA.tile([128, 512], F32, tag="ps",
                                          name=f"sc{l}_{h}_{s}_{tk}")
                            nc.tensor.matmul(
                                sc[:, :n], ka[0][h][:, tk * 128:(tk + 1) * 128],
                                qa[0][h][:, s * 512 + col0:(s + 1) * 512],
                                start=True, stop=True)
                            w4 = w4pool.tile([128, 512], FP16, tag="w4")
                            sigpipe(4 * i + h, spool, sc, w4, n, masked)
                            ring[(h, i)] = (w4, col0, n, tk)
                    j = i - LAG
                    if 0 <= j < nrounds:
                        for h in range(HPC):
                            w4, col0, n, tk = ring.pop((h, j))
                            nc.tensor.matmul(
                                avs[h][:, col0:],
                                vaug[0][:, tk, h * 65:(h + 1) * 65],
                                w4[:, :n], start=(j == 0),
                                stop=(j == nrounds - 1),
                                skip_group_check=True)
                    yield
                # normalization: divide by accumulated denominator row
                for h in range(HPC):
                    dd = rowpool.tile([1, 512], F32, tag="row")
                    nc.vector.tensor_scalar(dd[:], avs[h][64:65, :],
                                            scalar1=16.0 * EPS, scalar2=None,
                                            op0=ALU.add, op1=ALU.bypass)
                    dr = rowpool.tile([1, 512], F32, tag="row")
                    nc.vector.reciprocal_approx_fast(dr[:], dd[:])
                    drb = rowpool.tile([1, 512], FP16, tag="rowb")
                    nc.vector.tensor_copy(drb[:], dr[:])
                    reps = reppool.tile([64, 512], FP16, tag="reps")
                    nc.gpsimd.partition_broadcast(reps[:], drb[:])
                    pair, half = h // 2, h % 2
                    nc.vector.tensor_tensor(
                        asb[0][64 * half:64 * (half + 1), pair,
                               s * 512:(s + 1) * 512],
                        avs[h][0:64, :], reps[:], ALU.mult)
                    if h % 2 == 1:
                        yield

            def units_outproj(l, s, idx, wo):
                sl = slice(s * 512, (s + 1) * 512)
                for ot in range(DT):
                    ps = psA.tile([128, 512], F32, tag="ps")
                    for p in range(2):
                        nc.tensor.matmul(ps[:],
                                         wo[:, p, ot * 128:(ot + 1) * 128],
                                         asb[0][:, p, sl], start=(p == 0),
                                         stop=(p == 1))
                    dl1 = dlpool.tile([128, 512], FP16, tag="dl1",
                                      name=f"dla{l}_{s}_{ot}")
                    if ot % 2 == 0:
                        nc.scalar.activation(dl1[:], ps[:], AF.Copy, scale=1.0)
                    else:
                        nc.vector.tensor_copy(dl1[:], ps[:])
                    eng = nc.sync if ot % 2 == 0 else nc.scalar
                    eng.dma_start(cc_in[idx][ot * 128:(ot + 1) * 128, :],
                                  dl1[:])
                    yield
                nc.gpsimd.collective_compute(
                    "AllReduce", ALU.add, ins=[cc_in[idx][:]],
                    outs=[cc_out[idx][:]], replica_groups=RG)

            def units_ffn_in(l, s, wm):
                hsb = hsbpool.tile([128, NFT_FF, 512], FP16, tag="hsb",
                                   name=f"hsb{l}_{s}")
                hsbs[s] = hsb
                sl = slice(s * 512, (s + 1) * 512)
                for ft in range(NFT_FF):
                    gps = psA.tile([128, 512], F32, tag="ps")
                    vps = psA.tile([128, 512], F32, tag="ps")
                    for dt in range(DT):
                        nc.tensor.matmul(gps[:],
                                         wm[:, dt, ft * 128:(ft + 1) * 128],
                                         xn[:, dt, sl], start=(dt == 0),
                                         stop=(dt == DT - 1))
                    for dt in range(DT):
                        nc.tensor.matmul(
                            vps[:],
                            wm[:, dt,
                               DFF_SH + ft * 128:DFF_SH + (ft + 1) * 128],
                            xn[:, dt, sl], start=(dt == 0),
                            stop=(dt == DT - 1))
                    # h = g*(1+u)*v with u = g/(1+|g|)  (x0.5 folded into w3)
                    # psum tiles are freed by their FIRST consumer: gb/a read
                    # gps right away, vb reads vps right away.
                    gb = spool.tile([128, 512], FP16, tag="gb")
                    nc.vector.tensor_copy(gb[:], gps[:])
                    a = spool.tile([128, 512], FP16, tag="aa")
                    nc.scalar.activation(a[:], gps[:], AF.Abs, scale=1.0)
                    vb = spool.tile([128, 512], FP16, tag="vb")
                    nc.vector.tensor_copy(vb[:], vps[:])
                    if ft % 2 == 0:
                        ln = spool.tile([128, 512], FP16, tag="rr")
                        nc.scalar.activation(ln[:], a[:], AF.Ln, bias=1.0,
                                             scale=1.0)
                        r = spool.tile([128, 512], FP16, tag="rb")
                        nc.scalar.activation(r[:], ln[:], AF.Exp, scale=-1.0)
                    else:
                        dte = spool.tile([128, 512], F32, tag="df")
                        nc.vector.tensor_scalar(dte[:], a[:], scalar1=1.0,
                                                scalar2=None, op0=ALU.add,
                                                op1=ALU.bypass)
                        r = spool.tile([128, 512], F32, tag="rf")
                        nc.vector.reciprocal_approx_fast(r[:], dte[:])
                    u = spool.tile([128, 512], FP16, tag="uu")
                    nc.vector.tensor_tensor(u[:], gb[:], r[:], ALU.mult)
                    t = spool.tile([128, 512], FP16, tag="tt")
                    nc.vector.tensor_scalar(t[:], u[:], scalar1=1.0,
                                            scalar2=None, op0=ALU.add,
                                            op1=ALU.bypass)
                    m2 = spool.tile([128, 512], FP16, tag="m2")
                    if ft % 2 == 0:
                        nc.gpsimd.tensor_tensor(m2[:], gb[:], t[:], ALU.mult)
                        nc.vector.tensor_tensor(hsb[:, ft], m2[:], vb[:],
                                                ALU.mult)
                    else:
                        nc.vector.tensor_tensor(m2[:], gb[:], t[:], ALU.mult)
                        nc.gpsimd.tensor_tensor(hsb[:, ft], m2[:], vb[:],
                                                ALU.mult)
                    yield

            def units_ffn_out(l, s, idx, w3, hsb):
                for ot in range(DT):
                    ps = psA.tile([128, 512], F32, tag="ps")
                    for ft in range(NFT_FF):
                        nc.tensor.matmul(ps[:],
                                         w3[:, ft, ot * 128:(ot + 1) * 128],
                                         hsb[:, ft], start=(ft == 0),
                                         stop=(ft == NFT_FF - 1))
                    dl1 = dlpool.tile([128, 512], FP16, tag="dl1",
                                      name=f"dlm{l}_{s}_{ot}")
                    if ot % 2 == 0:
                        nc.scalar.activation(dl1[:], ps[:], AF.Copy, scale=1.0)
                    else:
                        nc.vector.tensor_copy(dl1[:], ps[:])
                    eng = nc.sync if ot % 2 == 0 else nc.scalar
                    eng.dma_start(cc_in[idx][ot * 128:(ot + 1) * 128, :],
                                  dl1[:])
                    yield
                nc.gpsimd.collective_compute(
                    "AllReduce", ALU.add, ins=[cc_in[idx][:]],
                    outs=[cc_out[idx][:]], replica_groups=RG)

            def drain(*gens):
                for g in gens:
                    for _ in g:
                        pass

            def seq(*gens):
                for g in gens:
                    yield from g

            # ================= the schedule (v4) ======================
            # Every AllReduce gets >=30us of emitted work between fire and
            # consume; scores-s1 splits in half around addnorm(a0) so the
            # second half can interleave ffn_in(s0) as PE filler.
            load_qkvo(0)
            load_wm(0)
            load_w3(0)
            pending = []        # deferred addnorm(f1) of previous layer
            prev_ffn_out = None
            for l in range(L):
                alloc_attn(l)
                # U1: qkv strip 0, half of prev-layer ffn_out s1 interleaved
                u1 = seq(units_qk(l, 0), units_v(l, 0))
                for k, _ in enumerate(u1):
                    if prev_ffn_out is not None and k % 2 == 0:
                        next(prev_ffn_out, None)
                # U2: scores s0 + rest of prev ffn_out (AR f1 fires inside)
                sc0 = units_scores(l, 0)
                for _ in sc0:
                    if prev_ffn_out is not None:
                        next(prev_ffn_out, None)
                if prev_ffn_out is not None:
                    drain(prev_ffn_out)
                    prev_ffn_out = None
                if l >= 1:
                    load_w3(l)
                # U3: outproj s0 -> AR a0; prev addnorm(f1) lands inside
                op0 = units_outproj(l, 0, 4 * l, wt["wo"])
                for k, _ in enumerate(op0):
                    if k == 3 and pending:
                        pending.pop()()
                if pending:
                    pending.pop()()
                # U4: qkv strip 1 (needs xn s1 from addnorm f1)
                drain(units_qk(l, 1), units_v(l, 1))
                # U5: scores s1 halves around addnorm(a0); ffn_in(s0)
                # interleaves into the second half
                ff_in0 = units_ffn_in(l, 0, wt["wm"])
                sc1 = units_scores(l, 1)
                for i, _ in enumerate(sc1):
                    if i == 3:
                        addnorm(4 * l, 0)          # consume AR a0
                    elif i >= 5:
                        next(ff_in0, None)
                # U6: outproj s1 -> AR a1, interleaved with rest of ffn_in0
                op1 = units_outproj(l, 1, 4 * l + 1, wt["wo"])
                for _ in op1:
                    next(ff_in0, None)
                drain(ff_in0)
                if l + 1 < L:
                    load_qkvo(l + 1)
                # U7: ffn_out s0 -> AR f0; addnorm(a1) at unit 5
                ff_out0 = units_ffn_out(l, 0, 4 * l + 2, wt["w3"], hsbs[0])
                for k, _ in enumerate(ff_out0):
                    if k == 5:
                        addnorm(4 * l + 1, 1)      # consume AR a1
                # U8: ffn_in s1; addnorm(f0) at the end
                ff_in1 = units_ffn_in(l, 1, wt["wm"])
                drain(ff_in1)
                addnorm(4 * l + 2, 0)              # consume AR f0
                # U9: ffn_out s1 (deferred into next layer's U1/U2)
                prev_ffn_out = units_ffn_out(l, 1, 4 * l + 3, wt["w3"],
                                             hsbs[1])
                pending = [lambda idx=4 * l + 3: addnorm(idx, 1)]
                if l + 1 < L:
                    load_wm(l + 1)
            # final layer: drain the deferred ffn_out s1 before lm_head
            drain(prev_ffn_out)

        # ================= lm_head ========================================
        with tc.tile_pool(name="lmw", bufs=NVS) as lmw, \
             tc.tile_pool(name="lms", bufs=4) as lms:
            wts = {}

            def lm_fetch(vs):
                vw = min(512, VOC_SH - vs * 512)
                wtile = lmw.tile([128, DT, 512], FP16, tag="wemb",
                                 name=f"wemb{vs}")
                eng = nc.sync if vs % 2 == 0 else nc.scalar
                eng.dma_start(
                    wtile[:, :, :vw], membT[:, vs * 512:vs * 512 + vw]
                    .rearrange("(dt p) f -> p dt f", p=128))
                wts[vs] = wtile

            def lm_tile(vs, tt):
                vw = min(512, VOC_SH - vs * 512)
                wtile = wts[vs]
                ps = psA.tile([128, 512], F32, tag="ps")
                for dt in range(DT):
                    nc.tensor.matmul(ps[:, :vw],
                                     xn[:, dt, tt * 128:(tt + 1) * 128],
                                     wtile[:, dt, :vw],
                                     start=(dt == 0), stop=(dt == DT - 1))
                ls = lms.tile([128, 512], FP16, tag="lmsb")
                if tt % 2 == 0:
                    nc.scalar.activation(ls[:, :vw], ps[:, :vw], AF.Copy,
                                         scale=1.0)
                else:
                    nc.vector.tensor_copy(ls[:, :vw], ps[:, :vw])
                eng = nc.sync if tt % 2 == 0 else nc.scalar
                eng.dma_start(
                    logits[tt * 128:(tt + 1) * 128,
                           vs * 512:vs * 512 + vw],
                    ls[:, :vw])

            lm_fetch(0)
            lm_fetch(1)
            # pass A: strip-0 token tiles of every block; the final
            # AR-dependent addnorm lands after a few blocks of cover
            for vs in range(NVS):
                if vs + 2 < NVS:
                    lm_fetch(vs + 2)
                for tt in range(4):
                    lm_tile(vs, tt)
                if vs == 6 and pending:
                    pending.pop()()
            # pass B: strip-1 token tiles (weights still resident)
            for vs in range(NVS):
                for tt in range(4, DT):
                    lm_tile(vs, tt)
    nc.compile()
    return nc


def _prep_inputs(input_ids, emb, qkv_w, out_w, n1_w, n2_w, wm_w, w3_w, fn_w):
    ids = np.asarray(input_ids)
    emb = np.asarray(emb, dtype=np.float32)
    x0 = emb[ids]                                   # [B, T, D]
    mag = np.mean(np.abs(x0), axis=-1, keepdims=True)
    xn0 = x0 / (mag + EPS)
    iota = np.arange(T, dtype=np.float32)
    qkv_w = np.asarray(qkv_w, dtype=np.float32)
    out_w = np.asarray(out_w, dtype=np.float32)
    wm_w = np.asarray(wm_w, dtype=np.float32)
    w3_w = np.asarray(w3_w, dtype=np.float32)
    n1_w = np.asarray(n1_w, dtype=np.float32)
    n2_w = np.asarray(n2_w, dtype=np.float32)
    fn_w = np.asarray(fn_w, dtype=np.float32)
    per_core = []
    for c in range(NCORES):
        b, r = c // TP, c % TP
        heads = list(range(HPC * r, HPC * r + HPC))
        qa = np.stack([np.stack([-iota, np.full(T, ALIBI[h], np.float32)])
                       for h in heads]).astype(np.float32)
        ka = np.stack([np.stack([np.full(T, ALIBI[h], np.float32), iota])
                       for h in heads]).astype(np.float32)
        wqk = np.empty((L, D, FSH), np.float32)
        wv = np.empty((L, D, VSH), np.float32)
        wo = np.empty((L, VSH, D), np.float32)
        wm = np.zeros((L, D, 2 * DFF_SH), np.float32)
        w3 = np.zeros((L, DFF_SH, D), np.float32)
        for l in range(L):
            q3 = qkv_w[l].reshape(3, H, DH, D)
            qrows = q3[0, heads].reshape(VSH, D) * SCALE
            krows = q3[1, heads].reshape(VSH, D)
            vrows = q3[2, heads].reshape(VSH, D)
            n1 = n1_w[l][:, None]                   # fold into d-rows of W^T
            wqk[l] = np.concatenate([qrows, krows], 0).T * n1
            wv[l] = vrows.T * n1
            ow = out_w[l].reshape(D, H, DH)[:, heads].reshape(D, VSH)
            wo[l] = ow.T
            n2 = n2_w[l][:, None]
            g0, g1 = DFF_SH * r, min(DFF_SH * (r + 1), DFF)
            ng = g1 - g0
            if ng > 0:
                wm[l, :, :ng] = wm_w[l][g0:g1].T * n2
                wm[l, :, DFF_SH:DFF_SH + ng] = wm_w[l][DFF + g0:DFF + g1].T * n2
                w3[l, :ng] = 0.5 * w3_w[l][:, g0:g1].T
        memb = (emb[VOC_SH * r:VOC_SH * (r + 1)] * fn_w[None, :]).T
        per_core.append(dict(
            x0T=np.ascontiguousarray(x0[b].T),
            xn0T=np.ascontiguousarray(xn0[b].T).astype(np.float16),
            qaug=qa, kaug=ka,
            wqkT=np.ascontiguousarray(wqk).astype(np.float16),
            wvT=np.ascontiguousarray(wv).astype(np.float16),
            woT=np.ascontiguousarray(wo).astype(np.float16),
            wmT=np.ascontiguousarray(wm).astype(np.float16),
            w3T=np.ascontiguousarray(w3).astype(np.float16),
            membT=np.ascontiguousarray(memb).astype(np.float16),
        ))
    return per_core


def kernel(**inputs):
    if "nc" not in _CACHE:
        _CACHE["nc"] = build_nc()
    nc = _CACHE["nc"]
    per_core = _prep_inputs(**inputs)
    res = run_bass_kernel_spmd(nc, per_core, core_ids=list(range(NCORES)),
                               **_CACHE.get("run_kwargs", {}))
    _CACHE["last_result"] = res
    out = np.empty((B, T, V), np.float32)
    for c in range(NCORES):
        b, r = c // TP, c % TP
        out[b, :, VOC_SH * r:VOC_SH * (r + 1)] = \
            res.results[c]["logits"].astype(np.float32)
    return out
